# revision 1
# baseline (speedup 1.0000x reference)
"""ConvModLayer (StyleGAN2-style modulated 3x3 conv) on 8 Trainium2
NeuronCores — data-parallel over batch (16 samples -> 2 per core),
computed via Winograd F(2x2,3x3) in bf16.

Math (equivalent to the reference):
  cscale = 1/sqrt(512*9)   (folded into the host-transformed weights)
  sigma_sq[b,o] = sum_i wsq[i,o] * s[b,i]^2,  wsq[i,o] = sum_k (w*cscale)^2
  out[b] = untransform( sum_i W_wino[pos,i,o] * V[pos,i,tile] ) * rsqrt(sigma)

Winograd pipeline per core (2 samples x 4 quarter-image blocks of 8
tile-rows; 16 positions = (ry,rx) in 4x4):
  host:   W_wino = G w G^T (bf16), x pre-padded + 2x2-parity-split (bf16)
  DVE:    per-channel style scale of x (4x-mode tensor_scalar),
          y-transform (stage1) + x-transform (stage2) -> V bf16,
          x-untransform (stage A) + partial y-untransform
  PE:     16 pos x 4 ic_chunk x 4 oc_chunk bf16 matmuls -> PSUM f32
  ACT:    PSUM->SBUF copy fused with rsqrt(sigma) channel scale (bf16)
  GPSIMD: final y-untransform writes, f32 interleaved into image layout
  DMA out per (sample, quarter, oc_chunk)
"""

import sys
from contextlib import ExitStack

if "/opt/trn_rl_repo" not in sys.path:
    sys.path.insert(0, "/opt/trn_rl_repo")

import numpy as np
import ml_dtypes

import concourse.bacc as bacc
import concourse.mybir as mybir
import concourse.tile as tile
from concourse.bass_utils import run_bass_kernel_spmd

F32 = mybir.dt.float32
BF16 = mybir.dt.bfloat16
BF = ml_dtypes.bfloat16

N_CORES = 8
B = 16
B2 = B // N_CORES
C = 512
NCH = 4
H = W = 64
EPS = 1e-8
CSCALE = 1.0 / (C * 9) ** 0.5

_NC_CACHE = {}


def _build(inplace_scale: bool = True):
    nc = bacc.Bacc("TRN2", target_bir_lowering=False, debug=False)

    # x4: padded image split by (row-parity, col-parity); par = rp*2+cp
    x4_d = nc.dram_tensor("x4", [B2, NCH, 128, 4, 33, 33], BF16,
                          kind="ExternalInput")
    s_d = nc.dram_tensor("s", [128, NCH, B2], F32, kind="ExternalInput")
    w_d = nc.dram_tensor("w", [128, 16, NCH, C], BF16, kind="ExternalInput")
    wsq_d = nc.dram_tensor("wsq", [128, NCH, C], BF16, kind="ExternalInput")
    o_d = nc.dram_tensor("o", [B2, NCH, 128, H, W], F32, kind="ExternalOutput")

    with tile.TileContext(nc) as tc, ExitStack() as ctx:
        wpool = ctx.enter_context(tc.tile_pool(name="wpool", bufs=1))
        spool = ctx.enter_context(tc.tile_pool(name="spool", bufs=1))
        x4pool = ctx.enter_context(tc.tile_pool(name="x4pool", bufs=1))
        ipool = ctx.enter_context(tc.tile_pool(name="ipool", bufs=1))
        vpool = ctx.enter_context(tc.tile_pool(name="vpool", bufs=2))
        mtpool = ctx.enter_context(tc.tile_pool(name="mtpool", bufs=2))
        zpool = ctx.enter_context(tc.tile_pool(name="zpool", bufs=3))
        upool = ctx.enter_context(tc.tile_pool(name="upool", bufs=2))
        outpool = ctx.enter_context(tc.tile_pool(name="outpool", bufs=2))
        pspool = ctx.enter_context(
            tc.tile_pool(name="pspool", bufs=2, space="PSUM")
        )

        # ---- style + sigma ----
        s_t = spool.tile([128, NCH, B2], F32)
        nc.sync.dma_start(s_t[:], s_d[:])
        wsq_t, wsq_free = tc.tile([128, NCH, C], BF16, name="wsq_t")
        nc.sync.dma_start(wsq_t[:], wsq_d[:])

        s2f_t = spool.tile([128, NCH, B2], F32)
        nc.vector.tensor_mul(s2f_t[:], s_t[:], s_t[:])
        s2_t = spool.tile([128, NCH, B2], BF16)
        nc.vector.tensor_copy(s2_t[:], s2f_t[:])

        psig = pspool.tile([128, NCH, B2], F32, tag="ps", name="psig")
        for oc in range(NCH):
            for ic in range(NCH):
                nc.tensor.matmul(
                    psig[:, oc, :],
                    wsq_t[:, ic, oc * 128:(oc + 1) * 128],
                    s2_t[:, ic, :],
                    start=(ic == 0),
                    stop=(ic == 3),
                )
        rsig_t = spool.tile([128, NCH, B2], F32)
        nc.vector.tensor_scalar_add(rsig_t[:], psig[:], EPS)
        nc.scalar.sqrt(rsig_t[:], rsig_t[:])
        nc.vector.reciprocal(rsig_t[:], rsig_t[:])
        wsq_free()

        # ---- weight DMAs (emitted in two halves; interleaved below) ----
        w_t = wpool.tile([128, 16, NCH, C], BF16)

        def emit_w(lo, hi):
            nc.sync.dma_start(w_t[:, lo:hi], w_d[:, lo:hi])

        # ---- per-quarter input chain: DMA -> scale -> stage1 -> stage2 ----
        def input_chain(b, q):
            t0 = 8 * q
            x4_t = x4pool.tile([128, NCH, 4, 9, 33], BF16, tag="x4", name="x4")
            for ic in range(NCH):
                nc.sync.dma_start(
                    x4_t[:, ic], x4_d[b, ic, :, :, t0:t0 + 9, :]
                )
            # style scale (ACT, which has headroom), in place
            for ic in range(NCH):
                nc.scalar.mul(
                    x4_t[:, ic], x4_t[:, ic], s_t[:, ic, b:b + 1]
                )
            # stage1 (y-transform): I[cp, ic, ry*8+ty, u], both col-parities
            # in one op per ry (out AP transposed to (ic, cp, ty, u))
            i_t = ipool.tile([128, 2, NCH, 32, 33], BF16, tag="i", name="i_t")
            e0 = x4_t[:, :, 0:2, 0:8, :]  # rows 2t   (rp=0), cp 0..1
            e1 = x4_t[:, :, 0:2, 1:9, :]  # rows 2t+2
            o0 = x4_t[:, :, 2:4, 0:8, :]  # rows 2t+1 (rp=1)
            o1 = x4_t[:, :, 2:4, 1:9, :]  # rows 2t+3

            def iout(ry):
                return i_t[:, :, :, ry * 8:(ry + 1) * 8, :].transpose(
                    [0, 2, 1, 3, 4]
                )

            nc.vector.tensor_sub(iout(0), e0, e1)
            nc.vector.tensor_add(iout(1), o0, e1)
            nc.vector.tensor_sub(iout(2), e1, o0)
            nc.vector.tensor_sub(iout(3), o0, o1)
            # stage2 (x-transform): V[rx, ic, ry*8+ty, tx]
            v_t = vpool.tile([128, 4, NCH, 32, 32], BF16, tag="v", name="v_t")
            ie0 = i_t[:, 0, :, :, 0:32]
            ie1 = i_t[:, 0, :, :, 1:33]
            io0 = i_t[:, 1, :, :, 0:32]
            io1 = i_t[:, 1, :, :, 1:33]
            nc.vector.tensor_sub(v_t[:, 0], ie0, ie1)
            nc.vector.tensor_add(v_t[:, 1], io0, ie1)
            nc.vector.tensor_sub(v_t[:, 2], ie1, io0)
            nc.gpsimd.tensor_sub(v_t[:, 3], io0, io1)
            return v_t

        # ---- per-quarter compute chain: matmuls -> copy -> untransform ----
        def compute_chain(b, q, v_t):
            for oc in range(NCH):
                mt_t = mtpool.tile([128, 4, 4, 8, 32], BF16, tag="mt",
                                   name="mt")
                for ryp in range(2):
                    ps = pspool.tile([128, 2, 4, 8, 32], F32, tag="ps",
                                     name="ps")
                    for ry2 in range(2):
                        ry = 2 * ryp + ry2
                        for rx in range(4):
                            pos = 4 * ry + rx
                            for ic in range(NCH):
                                nc.tensor.matmul(
                                    ps[:, ry2, rx],
                                    w_t[:, pos, ic, oc * 128:(oc + 1) * 128],
                                    v_t[:, rx, ic, ry * 8:(ry + 1) * 8, :],
                                    start=(ic == 0),
                                    stop=(ic == 3),
                                )
                    # PSUM -> SBUF bf16, fused rsqrt(sigma) scale
                    nc.scalar.mul(
                        mt_t[:, 2 * ryp:2 * ryp + 2], ps[:],
                        rsig_t[:, oc, b:b + 1],
                    )
                # stage A (x-untransform), stacked over ry; ta=M0+M1 and
                # tb=M2+M3 fused into one op via step-2 rx slicing
                tab = upool.tile([128, 4, 2, 8, 32], BF16, tag="ta",
                                 name="tab")
                z_t = zpool.tile([128, 4, 2, 8, 32], BF16, tag="z", name="z")
                nc.vector.tensor_add(
                    tab[:], mt_t[:, :, 0:4:2], mt_t[:, :, 1:4:2]
                )
                nc.vector.tensor_add(z_t[:, :, 0], tab[:, :, 0],
                                     mt_t[:, :, 2])
                nc.vector.tensor_sub(z_t[:, :, 1], mt_t[:, :, 1],
                                     tab[:, :, 1])
                # stage B (y-untransform): u/t3 on DVE, finals on GPSIMD
                u_t = upool.tile([128, 2, 8, 32], BF16, tag="u", name="u")
                t3_t = upool.tile([128, 2, 8, 32], BF16, tag="t3", name="t3")
                nc.vector.tensor_add(u_t[:], z_t[:, 0], z_t[:, 1])
                nc.vector.tensor_sub(t3_t[:], z_t[:, 1], z_t[:, 2])
                out_t = outpool.tile([128, 8, 2, 32, 2], F32, tag="out",
                                     name="out")
                # transposed out APs: dims (p, ty, tx), one op per q-parity
                oq0 = out_t[:, :, 0, :, :].transpose([0, 3, 1, 2])
                oq1 = out_t[:, :, 1, :, :].transpose([0, 3, 1, 2])
                nc.gpsimd.tensor_add(oq0, u_t[:], z_t[:, 2])
                nc.gpsimd.tensor_sub(oq1, t3_t[:], z_t[:, 3])
                nc.sync.dma_start(
                    o_d[b, oc, :, 16 * q:16 * q + 16, :], out_t[:]
                )

        # ---- software-pipelined emission ----
        quarters = [(b, q) for b in range(B2) for q in range(4)]
        v_prev = None
        for idx, (b, q) in enumerate(quarters):
            v_cur = input_chain(b, q)
            if idx == 0:
                emit_w(0, 8)
            elif idx == 1:
                emit_w(8, 16)
            if v_prev is not None:
                compute_chain(*quarters[idx - 1], v_prev)
            v_prev = v_cur
        compute_chain(*quarters[-1], v_prev)

    nc.compile()
    return nc


def get_nc(**kwargs):
    key = tuple(sorted(kwargs.items()))
    if key not in _NC_CACHE:
        _NC_CACHE[key] = _build(**kwargs)
    return _NC_CACHE[key]


def _host_prep(weight):
    """Winograd weight transform + squared-weight table (host, once)."""
    G = np.array([[1, 0, 0], [0.5, 0.5, 0.5], [0.5, -0.5, 0.5], [0, 0, 1]],
                 dtype=np.float64)
    wc = weight.astype(np.float64) * CSCALE
    w4 = np.einsum("ab,oibc,dc->oiad", G, wc, G)  # [o, i, ry, rx]
    # device layout [128=i_inner, pos=ry*4+rx, ic_chunk, o]
    w_dev = np.ascontiguousarray(
        w4.reshape(C, NCH, 128, 4, 4).transpose(2, 3, 4, 1, 0).reshape(
            128, 16, NCH, C
        )
    ).astype(BF)
    wsq = (wc ** 2).sum(axis=(2, 3)).T  # [i, o]
    wsq_dev = np.ascontiguousarray(
        wsq.reshape(NCH, 128, C).transpose(1, 0, 2)
    ).astype(BF)
    return w_dev, wsq_dev


def make_in_maps(x, s, weight):
    x = np.asarray(x, dtype=np.float32)
    s = np.asarray(s, dtype=np.float32)
    weight = np.asarray(weight, dtype=np.float32)

    w_dev, wsq_dev = _host_prep(weight)

    # padded image, parity-split: x4[b, ic, p, rp*2+cp, t, u]
    #   = xpad[b, ic*128+p, 2t+rp, 2u+cp]
    xpad = np.zeros((B, C, H + 2, W + 2), np.float32)
    xpad[:, :, 1:-1, 1:-1] = x
    x4 = np.empty((B, C, 4, 33, 33), dtype=BF)
    x4[:, :, 0] = xpad[:, :, 0::2, 0::2]
    x4[:, :, 1] = xpad[:, :, 0::2, 1::2]
    x4[:, :, 2] = xpad[:, :, 1::2, 0::2]
    x4[:, :, 3] = xpad[:, :, 1::2, 1::2]
    x4 = x4.reshape(B, NCH, 128, 4, 33, 33)

    in_maps = []
    for core in range(N_CORES):
        bsl = slice(core * B2, (core + 1) * B2)
        ss = np.ascontiguousarray(
            s[bsl].reshape(B2, NCH, 128).transpose(2, 1, 0)
        )
        in_maps.append({
            "x4": np.ascontiguousarray(x4[bsl]),
            "s": ss,
            "w": w_dev,
            "wsq": wsq_dev,
        })
    return in_maps


def kernel(x, s, weight):
    nc = get_nc()
    in_maps = make_in_maps(x, s, weight)
    res = run_bass_kernel_spmd(nc, in_maps, list(range(N_CORES)))
    out = np.concatenate(
        [r["o"].reshape(B2, C, H, W) for r in res.results], axis=0
    )
    return out.astype(np.float32)

